# revision 7
# baseline (speedup 1.0000x reference)
"""Trainium2 Bass kernel for nn_CAM (GNN message passing, 8-core SPMD).

Strategy (per core i of 8, owning node rows R_i = [1024*i, 1024*(i+1))):
  - Host ships the TRANSPOSED column-block of each adjacency:
    adjT_x[:, R_i] (shape [8192, 1024]) so the TensorE contraction axis
    (all 8192 source nodes) lands on SBUF partitions. fp32, streamed from
    HBM exactly once (adjT_f and adjT_s on the sync HWDGE ring; x/W1 on
    the scalar HWDGE ring so the two streams don't serialize).
  - The blend  con = meta*A_f + (1-meta)*A_s  is algebraically refactored
    as  con = (1-meta) * R  with  R = c*A_f + A_s,  c = meta/(1-meta).
    R is formed chunk-by-chunk with ONE fused DVE op
    (scalar_tensor_tensor: (A_f * c) + A_s -> bf16) and kept RESIDENT in
    SBUF (16 MB), so rounds 2 and 3 re-read it from SBUF instead of HBM.
    The (1-meta) factor is folded into the support matrices right before
    each AllGather, so no extra work on the big matrices is ever done.
  - Support matrices are computed in the transposed domain
    (z^T = s-stationary matmuls with R^T as the bf16 moving operand),
    exchanged across cores with AllGather (bf16) between rounds.
  - The attention fusion runs entirely in the transposed [64, 1024]
    domain; the host transposes the tiny per-core outputs back.
"""

import sys

if "/opt/trn_rl_repo" not in sys.path:
    sys.path.insert(0, "/opt/trn_rl_repo")

from contextlib import ExitStack

import numpy as np

import concourse.bass as bass
import concourse.tile as tile
from concourse import bacc, mybir
from concourse.bass_utils import run_bass_kernel_spmd
from concourse.masks import make_identity

F32 = mybir.dt.float32
BF16 = mybir.dt.bfloat16
AF = mybir.ActivationFunctionType

N = 8192
D_IN = 3000
H1, H2, Z = 256, 128, 64
N_CORES = 8
NL = N // N_CORES           # 1024 local nodes per core
KC = N // 128               # 64 contraction chunks of 128
XC = (D_IN + 127) // 128    # 24 x-feature chunks (last partial: 56)
RG = [list(range(N_CORES))]


def _emit(nc, tc, io):
    """Emit the whole per-core program inside a TileContext.

    Tile pools are a stack allocator (strict LIFO, zones reused across
    sibling scopes -- a later pool's first use waits for the release of
    the pool whose zone it reuses). The A-phase staging pool is opened
    BEFORE the phase-0 pools so the adjacency stream starts at t=0.

    Engine queues are FIFO, so DMA issue placement matters:
      sync ring   : adjT_f all slabs + adjT_s slabs < ACT_SPLIT
      scalar ring : x / W1 slabs (early), then s1-chain ACT compute,
                    then adjT_s slabs >= ACT_SPLIT, then tail chains
      gpsimd      : consts + AllGather bounces (naturally serial)
    """
    adjT_f, adjT_s, xT = io["adjT_f"], io["adjT_s"], io["xT"]
    zfT, zsT = io["zfT"], io["zsT"]
    W1, W2, W3 = io["W1"], io["W2"], io["W3"]
    wl_W, mlp_W = io["wl_W"], io["mlp_W"]
    wl_b, mlp_b, meta = io["wl_b"], io["mlp_b"], io["meta"]
    outT = io["outT"]

    ACT_SPLIT = 16  # adjT_s slabs >= this index go on the scalar ring

    ctx = ExitStack()
    with ctx:
        const = ctx.enter_context(tc.tile_pool(name="const", bufs=1))
        dram = ctx.enter_context(tc.tile_pool(name="dram", bufs=1, space="DRAM"))

        # ---- constants ----
        # meta comes first on the sync ring so the blend scalar is ready
        # before the first adjacency slab lands.
        meta_sb = const.tile([128, 1], F32)
        nc.sync.dma_start(out=meta_sb, in_=meta.ap().to_broadcast((128, 1)))
        om_sb = const.tile([128, 1], F32)    # 1 - meta
        nc.scalar.activation(om_sb, meta_sb, AF.Copy, scale=-1.0, bias=1.0)
        rec_om = const.tile([128, 1], F32)   # 1 / (1 - meta)
        nc.vector.reciprocal(rec_om, om_sb)
        c_sb = const.tile([128, 1], F32)     # meta / (1 - meta)
        nc.vector.tensor_mul(c_sb, meta_sb, rec_om)

        ident_bf = const.tile([128, 128], BF16)
        make_identity(nc, ident_bf)
        wl_b_sb = const.tile([64, 1], F32)
        nc.gpsimd.dma_start(out=wl_b_sb, in_=wl_b[:, None])
        mlp_b_sb = const.tile([64, 1], F32)
        nc.gpsimd.dma_start(out=mlp_b_sb, in_=mlp_b[:, None])
        W2_sb = const.tile([128, 2, H2], BF16)
        nc.gpsimd.dma_start(out=W2_sb, in_=W2.rearrange("(b k) c -> k b c", b=2))
        W3_sb = const.tile([128, Z], BF16)
        nc.gpsimd.dma_start(out=W3_sb, in_=W3[:, :])

        # resident blended adjacency (transposed): R^T[k_part, k_chunk, m]
        conT = const.tile([128, KC, NL], BF16)

        # AG bounce buffers
        s1_in = dram.tile([NL, H1], BF16)
        s1_out = dram.tile([N, H1], BF16, addr_space="Shared")
        s2_in = dram.tile([NL, H2], BF16)
        s2_out = dram.tile([N, H2], BF16, addr_space="Shared")
        s3_in = dram.tile([NL, Z], BF16)
        s3_out = dram.tile([N, Z], BF16, addr_space="Shared")

        # ======== phases 0+A+B: s1, stream+blend+round1, s2 ===========
        with tc.tile_pool(name="tailB", bufs=1) as tailB, \
             tc.tile_pool(name="stage", bufs=2) as stage, \
             tc.tile_pool(name="psZ", bufs=1, space="PSUM") as psZ:
            z1sb = tailB.tile([128, 2, NL], BF16)
            s2T_bf = tailB.tile([128, NL], BF16)
            s2loc = tailB.tile([128, 8, H2], BF16)
            z1_ps = [psZ.tile([128, NL], F32, name=f"z1g{g}") for g in range(2)]

            # ---- phase 0: s1 = tanh(x @ W1) on local rows ----
            with tc.tile_pool(name="chain0", bufs=1) as chain0:
                s1T_bf = chain0.tile([128, 2, NL], BF16)
                s1loc = chain0.tile([128, 8, H1], BF16)
                with tc.tile_pool(name="xstage", bufs=2) as xstage, \
                     tc.tile_pool(name="psA", bufs=1, space="PSUM") as psA:
                    s1T_ps = [psA.tile([128, NL], F32, name=f"s1T{g}")
                              for g in range(2)]
                    for kx in range(XC):
                        kp = min(128, D_IN - kx * 128)
                        xsl = xstage.tile([128, NL], F32, name="xsl")
                        nc.scalar.dma_start(
                            out=xsl[:kp], in_=xT[kx * 128 : kx * 128 + kp, :]
                        )
                        w1t = xstage.tile([128, H1], F32, name="w1t")
                        nc.scalar.dma_start(
                            out=w1t[:kp], in_=W1[kx * 128 : kx * 128 + kp, :]
                        )
                        xbf = xstage.tile([128, NL], BF16, name="xbf")
                        nc.scalar.copy(xbf[:kp], xsl[:kp])
                        w1bf = xstage.tile([128, H1], BF16, name="w1bf")
                        nc.scalar.copy(w1bf[:kp], w1t[:kp])
                        for g in range(2):
                            for h in range(2):
                                nc.tensor.matmul(
                                    s1T_ps[g][:, h * 512 : (h + 1) * 512],
                                    lhsT=w1bf[:kp, g * 128 : (g + 1) * 128],
                                    rhs=xbf[:kp, h * 512 : (h + 1) * 512],
                                    start=(kx == 0),
                                    stop=(kx == XC - 1),
                                )
                    for g in range(2):
                        nc.scalar.activation(s1T_bf[:, g], s1T_ps[g], AF.Tanh)
                # transpose to node-major, scaled by (1-meta)
                with tc.tile_pool(name="psT", bufs=2, space="PSUM") as psT:
                    for mb in range(8):
                        for g in range(2):
                            tp = psT.tile([128, 128], BF16, name="tp")
                            nc.tensor.transpose(
                                tp, s1T_bf[:, g, mb * 128 : (mb + 1) * 128],
                                ident_bf,
                            )
                            nc.scalar.activation(
                                s1loc[:, mb, g * 128 : (g + 1) * 128], tp,
                                AF.Copy, scale=om_sb,
                            )
                nc.gpsimd.dma_start(
                    out=s1_in.rearrange("(a p) c -> p a c", p=128), in_=s1loc
                )
                nc.gpsimd.collective_compute(
                    "AllGather", mybir.AluOpType.bypass, replica_groups=RG,
                    ins=[s1_in.opt()], outs=[s1_out.opt()],
                )

            # ---- phase A: stream adjacency, blend, round 1 ----
            s1f_cur = None
            for j in range(KC // 2):  # 32 slabs x 2 chunks
                af = stage.tile([128, 2, NL], F32, name="af")
                nc.sync.dma_start(
                    out=af,
                    in_=adjT_f[j * 256 : (j + 1) * 256, :].rearrange(
                        "(a p) m -> p a m", p=128
                    ),
                )
                asl = stage.tile([128, 2, NL], F32, name="asl")
                as_eng = nc.sync if j < ACT_SPLIT else nc.scalar
                as_eng.dma_start(
                    out=asl,
                    in_=adjT_s[j * 256 : (j + 1) * 256, :].rearrange(
                        "(a p) m -> p a m", p=128
                    ),
                )
                if j % 4 == 0:
                    q = j // 4
                    s1f_cur = stage.tile([128, 8, H1], BF16, name="s1f", bufs=3)
                    nc.gpsimd.dma_start(
                        out=s1f_cur,
                        in_=s1_out[q * 1024 : (q + 1) * 1024, :].rearrange(
                            "(a p) c -> p a c", p=128
                        ),
                    )
                for t in range(2):
                    k = 2 * j + t
                    # blend on DVE: conT[k] = (A_f * c) + A_s  (bf16 out)
                    nc.vector.scalar_tensor_tensor(
                        out=conT[:, k, :],
                        in0=af[:, t, :],
                        scalar=c_sb,
                        in1=asl[:, t, :],
                        op0=mybir.AluOpType.mult,
                        op1=mybir.AluOpType.add,
                    )
                    for g in range(2):
                        for h in range(2):
                            sl = slice(h * 512, (h + 1) * 512)
                            nc.tensor.matmul(
                                z1_ps[g][:, sl],
                                lhsT=s1f_cur[:, k % 8, g * 128 : (g + 1) * 128],
                                rhs=conT[:, k, sl],
                                start=(k == 0),
                                stop=(k == KC - 1),
                            )
            # copy z1 out of PSUM (bf16) before psZ closes
            nc.vector.tensor_copy(z1sb[:, 0], z1_ps[0])
            nc.scalar.copy(z1sb[:, 1], z1_ps[1])

        # ---- phase B: s2 = tanh(z1 @ W2) ----
        with tc.tile_pool(name="tailB2", bufs=1) as tailB2:
            s2T_bf2 = tailB2.tile([128, NL], BF16)
            s2loc2 = tailB2.tile([128, 8, H2], BF16)
            with tc.tile_pool(name="psC", bufs=1, space="PSUM") as psC:
                s2T_ps = psC.tile([128, NL], F32)
                for b in range(2):
                    for h in range(2):
                        sl = slice(h * 512, (h + 1) * 512)
                        nc.tensor.matmul(
                            s2T_ps[:, sl], lhsT=W2_sb[:, b], rhs=z1sb[:, b, sl],
                            start=(b == 0), stop=(b == 1),
                        )
                nc.scalar.activation(s2T_bf2, s2T_ps, AF.Tanh)
            with tc.tile_pool(name="psT2", bufs=2, space="PSUM") as psT2:
                for mb in range(8):
                    tp = psT2.tile([128, 128], BF16, name="tp2")
                    nc.tensor.transpose(
                        tp, s2T_bf2[:, mb * 128 : (mb + 1) * 128], ident_bf
                    )
                    nc.scalar.activation(s2loc2[:, mb], tp, AF.Copy, scale=om_sb)
            nc.gpsimd.dma_start(
                out=s2_in.rearrange("(a p) c -> p a c", p=128), in_=s2loc2
            )
            nc.gpsimd.collective_compute(
                "AllGather", mybir.AluOpType.bypass, replica_groups=RG,
                ins=[s2_in.opt()], outs=[s2_out.opt()],
            )

        # ================= phase C: round 2 (z2 = R @ s2') =============
        with tc.tile_pool(name="tailC", bufs=1) as tailC:
            z2sb = tailC.tile([128, NL], BF16)
            s3T_bf = tailC.tile([64, NL], BF16)
            s3loc = tailC.tile([128, 8, Z], BF16)
            with tc.tile_pool(name="stageC", bufs=4) as stageC, \
                 tc.tile_pool(name="psD", bufs=1, space="PSUM") as psD:
                z2_ps = psD.tile([128, NL], F32)
                s2f = []
                for q in range(4):
                    s2f_q = stageC.tile([128, 16, H2], BF16, name="s2f")
                    nc.scalar.dma_start(
                        out=s2f_q,
                        in_=s2_out[q * 2048 : (q + 1) * 2048, :].rearrange(
                            "(a p) c -> p a c", p=128
                        ),
                    )
                    s2f.append(s2f_q)
                for k in range(KC):
                    for h in range(2):
                        sl = slice(h * 512, (h + 1) * 512)
                        nc.tensor.matmul(
                            z2_ps[:, sl], lhsT=s2f[k // 16][:, k % 16, :],
                            rhs=conT[:, k, sl],
                            start=(k == 0), stop=(k == KC - 1),
                        )
                nc.vector.tensor_copy(z2sb[:, :512], z2_ps[:, :512])
                nc.scalar.copy(z2sb[:, 512:], z2_ps[:, 512:])

            # s3 = z2 @ W3 (no activation); scale by (1-meta) in the copy
            with tc.tile_pool(name="psE", bufs=1, space="PSUM") as psE:
                s3T_ps = psE.tile([64, NL], F32)
                for h in range(2):
                    sl = slice(h * 512, (h + 1) * 512)
                    nc.tensor.matmul(s3T_ps[:, sl], lhsT=W3_sb, rhs=z2sb[:, sl])
                nc.scalar.copy(s3T_bf, s3T_ps)
            with tc.tile_pool(name="psT3", bufs=2, space="PSUM") as psT3:
                for mb in range(8):
                    tp = psT3.tile([128, 64], BF16, name="tp3")
                    nc.tensor.transpose(
                        tp, s3T_bf[:, mb * 128 : (mb + 1) * 128], ident_bf[:64, :64]
                    )
                    nc.scalar.activation(s3loc[:, mb], tp, AF.Copy, scale=om_sb)
            nc.gpsimd.dma_start(
                out=s3_in.rearrange("(a p) c -> p a c", p=128), in_=s3loc
            )
            nc.gpsimd.collective_compute(
                "AllGather", mybir.AluOpType.bypass, replica_groups=RG,
                ins=[s3_in.opt()], outs=[s3_out.opt()],
            )

        # ========= phases D+E: round 3 (com = R @ s3') + fusion ========
        with tc.tile_pool(name="tailD", bufs=1) as tailD:
            comT = tailD.tile([64, NL], F32)
            zfT_sb = tailD.tile([64, NL], F32)
            nc.gpsimd.dma_start(out=zfT_sb, in_=zfT[:, :])
            zsT_sb = tailD.tile([64, NL], F32)
            nc.gpsimd.dma_start(out=zsT_sb, in_=zsT[:, :])
            wlW_sb = tailD.tile([64, 64], F32)
            nc.gpsimd.dma_start(out=wlW_sb, in_=wl_W[:, :])
            mlpW_sb = tailD.tile([64, 3, 64], F32)
            nc.gpsimd.dma_start(
                out=mlpW_sb, in_=mlp_W.rearrange("(v c) d -> c v d", v=3)
            )
            with tc.tile_pool(name="stageD", bufs=4) as stageD, \
                 tc.tile_pool(name="psF", bufs=1, space="PSUM") as psF:
                com_ps = psF.tile([64, NL], F32)
                s3f = []
                for q in range(4):
                    s3f_q = stageD.tile([128, 16, Z], BF16, name="s3f")
                    nc.scalar.dma_start(
                        out=s3f_q,
                        in_=s3_out[q * 2048 : (q + 1) * 2048, :].rearrange(
                            "(a p) c -> p a c", p=128
                        ),
                    )
                    s3f.append(s3f_q)
                for k in range(KC):
                    for h in range(2):
                        sl = slice(h * 512, (h + 1) * 512)
                        nc.tensor.matmul(
                            com_ps[:, sl], lhsT=s3f[k // 16][:, k % 16, :],
                            rhs=conT[:, k, sl],
                            start=(k == 0), stop=(k == KC - 1),
                        )
                nc.vector.tensor_copy(comT, com_ps)

            # phase E: attention fusion (fp32)
            with tc.tile_pool(name="psG", bufs=2, space="PSUM") as psG:
                embs = [zfT_sb, comT, zsT_sb]
                aTs = [None, None, None]
                sqs = [None, None, None]
                for v in (0, 2, 1):  # com-dependent view last
                    a_ps = psG.tile([64, NL], F32, name="aps")
                    for h in range(2):
                        sl = slice(h * 512, (h + 1) * 512)
                        nc.tensor.matmul(a_ps[:, sl], lhsT=wlW_sb, rhs=embs[v][:, sl])
                    aT = tailD.tile([64, NL], F32, name=f"aT{v}")
                    nc.vector.tensor_scalar_add(aT, a_ps, wl_b_sb)
                    aTs[v] = aT
                    sqv = tailD.tile([64, NL], F32, name=f"sq{v}")
                    nc.scalar.activation(sqv, aT, AF.Square)
                    sqs[v] = sqv
                sq = tailD.tile([64, NL], F32)
                nc.vector.tensor_add(sq, sqs[0], sqs[2])
                nc.vector.tensor_add(sq, sq, sqs[1])
                nrm = tailD.tile([64, NL], F32)
                nc.scalar.activation(nrm, sq, AF.Sqrt)
                nc.vector.tensor_scalar_max(nrm, nrm, 1e-12)
                rec = tailD.tile([64, NL], F32)
                nc.vector.reciprocal_approx_fast(rec, nrm)

                out_ps = psG.tile([64, NL], F32, name="ops", bufs=1)
                for v in range(3):
                    u = tailD.tile([64, NL], F32, name="u", bufs=2)
                    nc.vector.tensor_mul(u, aTs[v], rec)
                    nc.vector.tensor_mul(u, u, embs[v])
                    for h in range(2):
                        sl = slice(h * 512, (h + 1) * 512)
                        nc.tensor.matmul(
                            out_ps[:, sl], lhsT=mlpW_sb[:, v], rhs=u[:, sl],
                            start=(v == 0), stop=(v == 2),
                        )
                outT_sb = tailD.tile([64, NL], F32)
                nc.vector.tensor_scalar_add(outT_sb, out_ps, mlp_b_sb)
                nc.gpsimd.dma_start(out=outT[:, :], in_=outT_sb)


_CACHE = {}


def _build():
    if "nc" in _CACHE:
        return _CACHE["nc"]
    nc = bacc.Bacc("TRN2", target_bir_lowering=False, debug=False,
                   num_devices=N_CORES)
    io = {
        "adjT_f": nc.dram_tensor("adjT_f", [N, NL], F32, kind="ExternalInput"),
        "adjT_s": nc.dram_tensor("adjT_s", [N, NL], F32, kind="ExternalInput"),
        "xT": nc.dram_tensor("xT", [D_IN, NL], F32, kind="ExternalInput"),
        "zfT": nc.dram_tensor("zfT", [Z, NL], F32, kind="ExternalInput"),
        "zsT": nc.dram_tensor("zsT", [Z, NL], F32, kind="ExternalInput"),
        "W1": nc.dram_tensor("W1", [D_IN, H1], F32, kind="ExternalInput"),
        "W2": nc.dram_tensor("W2", [H1, H2], F32, kind="ExternalInput"),
        "W3": nc.dram_tensor("W3", [H2, Z], F32, kind="ExternalInput"),
        "wl_W": nc.dram_tensor("wl_W", [Z, Z], F32, kind="ExternalInput"),
        "mlp_W": nc.dram_tensor("mlp_W", [3 * Z, Z], F32, kind="ExternalInput"),
        "wl_b": nc.dram_tensor("wl_b", [Z], F32, kind="ExternalInput"),
        "mlp_b": nc.dram_tensor("mlp_b", [Z], F32, kind="ExternalInput"),
        "meta": nc.dram_tensor("meta", [1], F32, kind="ExternalInput"),
        "outT": nc.dram_tensor("outT", [Z, NL], F32, kind="ExternalOutput"),
    }
    with tile.TileContext(nc) as tc:
        _emit(nc, tc, io)
    nc.compile()
    _CACHE["nc"] = nc
    return nc


def _shard_inputs(inputs):
    """Full inputs -> per-core input maps (host-side sharding only)."""
    f32 = np.float32
    adj_f = np.asarray(inputs["adj_feature"], f32)
    adj_s = np.asarray(inputs["adj_spatial"], f32)
    x = np.asarray(inputs["x"], f32)
    zf = np.asarray(inputs["z_feature"], f32)
    zs = np.asarray(inputs["z_spatial"], f32)
    rep = {
        "W1": np.ascontiguousarray(np.asarray(inputs["W1"], f32)),
        "W2": np.ascontiguousarray(np.asarray(inputs["W2"], f32)),
        "W3": np.ascontiguousarray(np.asarray(inputs["W3"], f32)),
        "wl_W": np.ascontiguousarray(np.asarray(inputs["wl_W"], f32)),
        "mlp_W": np.ascontiguousarray(np.asarray(inputs["mlp_W"], f32)),
        "wl_b": np.ascontiguousarray(np.asarray(inputs["wl_b"], f32)),
        "mlp_b": np.ascontiguousarray(np.asarray(inputs["mlp_b"], f32)),
        "meta": np.ascontiguousarray(np.asarray(inputs["meta"], f32)),
    }
    adj_fT = np.ascontiguousarray(adj_f.T)
    adj_sT = np.ascontiguousarray(adj_s.T)
    xT = np.ascontiguousarray(x.T)
    zfT = np.ascontiguousarray(zf.T)
    zsT = np.ascontiguousarray(zs.T)
    in_maps = []
    for i in range(N_CORES):
        r = slice(NL * i, NL * (i + 1))
        m = {
            "adjT_f": np.ascontiguousarray(adj_fT[:, r]),
            "adjT_s": np.ascontiguousarray(adj_sT[:, r]),
            "xT": np.ascontiguousarray(xT[:, r]),
            "zfT": np.ascontiguousarray(zfT[:, r]),
            "zsT": np.ascontiguousarray(zsT[:, r]),
        }
        m.update(rep)
        in_maps.append(m)
    return in_maps


def run(trace=False, **inputs):
    nc = _build()
    in_maps = _shard_inputs(inputs)
    res = run_bass_kernel_spmd(nc, in_maps, list(range(N_CORES)), trace=trace)
    out = np.concatenate(
        [np.asarray(res.results[i]["outT"]).T for i in range(N_CORES)], axis=0
    ).astype(np.float32)
    return out, res


def kernel(**inputs):
    out, _ = run(trace=False, **inputs)
    return out
